# revision 1
# baseline (speedup 1.0000x reference)
"""Category-specific MLP (MoE-style routing) on 8 Trainium2 NeuronCores.

Strategy (expert-ish data parallel, host-routed):
  - Host sorts the 64 samples by cat_id and assigns 8 consecutive sorted
    samples to each of the 8 cores (perfect token balance: 2048 tok/core).
  - Host gathers each core's per-sample weight banks W_l[cat] into a
    per-core DRAM input, so the device kernel is a uniform SPMD program:
    8 sample slots x 4 dense layers of [256,1024]x[1024,1024].
  - Activations live in transposed layout [D, tok] on chip; each layer is
    out_T = W_l.T @ h_T computed as matmul(lhsT=W tile, rhs=h_T tile), so
    layers chain on the tensor engine with no transposes. Host transposes
    x once on the way in and the output once on the way out.
"""

import numpy as np
from contextlib import ExitStack

import concourse.bass as bass
import concourse.mybir as mybir
import concourse.tile as tile
from concourse import bacc
from concourse.bass_utils import run_bass_kernel_spmd

P = 128          # SBUF partitions
D = 1024         # model dim (in = hidden = out)
KT = D // P      # 8 k-tiles per dim
TOK = 256        # tokens per sample
S = 8            # sample slots per core
L = 4            # layers
NCORES = 8

ACT_DT = mybir.dt.float32   # on-chip activation dtype
W_DT = mybir.dt.float32     # on-chip weight dtype
ACT_NP = np.float32
W_NP = np.float32

# Filled by kernel() with the BassKernelResults of the last run (for tests).
LAST_RESULT = None
_PROGRAM_CACHE = {}


def build_program(reps=1):
    """One SPMD program for all 8 cores: 8 slots x 4 layers.

    reps>1 wraps the whole computation in a hardware loop (only used for
    wall-clock slope timing in the test harness; grading uses reps=1).
    """
    nc = bacc.Bacc("TRN2", target_bir_lowering=False, debug=False,
                   num_devices=NCORES)
    xT = nc.dram_tensor("xT", [D, S * TOK], ACT_DT, kind="ExternalInput")
    wg = nc.dram_tensor("wg", [S, L, D, D], W_DT, kind="ExternalInput")
    bg = nc.dram_tensor("bg", [L, S, D], mybir.dt.float32, kind="ExternalInput")
    outT = nc.dram_tensor("outT", [D, S * TOK], mybir.dt.float32,
                          kind="ExternalOutput")

    xv = xT.ap().rearrange("(k p) n -> p k n", p=P)
    ov = outT.ap().rearrange("(k p) n -> p k n", p=P)
    bv = bg.ap().rearrange("l s (t p) -> p (l s t)", p=P)

    silu = mybir.ActivationFunctionType.Silu
    ident = mybir.ActivationFunctionType.Identity

    with tile.TileContext(nc) as tc, ExitStack() as ctx:
        wpool = ctx.enter_context(tc.tile_pool(name="w", bufs=3))
        hpool = ctx.enter_context(tc.tile_pool(name="h", bufs=3))
        opool = ctx.enter_context(tc.tile_pool(name="o", bufs=2))
        ppool = ctx.enter_context(tc.tile_pool(name="ps", bufs=6, space="PSUM"))
        cpool = ctx.enter_context(tc.tile_pool(name="c", bufs=1))

        btile = cpool.tile([P, L * S * KT], mybir.dt.float32)
        nc.sync.dma_start(btile[:], bv[:, :])

        def body(_iv=None):
            for s in range(S):
                hin = hpool.tile([P, KT, TOK], ACT_DT, tag="acts")
                nc.sync.dma_start(hin[:], xv[:, :, s * TOK:(s + 1) * TOK])
                for l in range(L):
                    w = wpool.tile([P, KT, D], W_DT, tag="w")
                    wsrc = wg.ap()[s, l].rearrange("(k p) m -> p k m", p=P)
                    for k in range(KT):
                        nc.sync.dma_start(w[:, k, :], wsrc[:, k, :])
                    last = l == L - 1
                    if last:
                        hout = opool.tile([P, KT, TOK], mybir.dt.float32,
                                          tag="outs")
                    else:
                        hout = hpool.tile([P, KT, TOK], ACT_DT, tag="acts")
                    for m in range(KT):
                        ps = ppool.tile([P, TOK], mybir.dt.float32)
                        for k in range(KT):
                            nc.tensor.matmul(ps[:], w[:, k, m * P:(m + 1) * P],
                                             hin[:, k, :],
                                             start=(k == 0), stop=(k == KT - 1))
                        col = (l * S + s) * KT + m
                        nc.scalar.activation(hout[:, m, :], ps[:],
                                             ident if last else silu,
                                             bias=btile[:, col:col + 1])
                    hin = hout
                nc.sync.dma_start(ov[:, :, s * TOK:(s + 1) * TOK], hin[:])

        if reps == 1:
            body()
        else:
            with tc.For_i(0, reps, 1) as iv:
                body(iv)
    nc.compile()
    return nc


def _routing(cat_ids):
    order = np.argsort(cat_ids, kind="stable")
    return order


def prepare_in_maps(x, cat_ids, Ws, bs, order):
    x = np.asarray(x)
    in_maps = []
    for c in range(NCORES):
        samp = order[c * S:(c + 1) * S]
        xs = np.asarray(x[samp], dtype=np.float32)          # [S, TOK, D]
        xTc = np.ascontiguousarray(xs.reshape(S * TOK, D).T)  # [D, S*TOK]
        cats = [int(cat_ids[i]) for i in samp]
        wgc = np.stack([np.stack([Ws[l][cat] for l in range(L)]) for cat in cats])
        bgc = np.stack([np.stack([bs[l][cat] for cat in cats]) for l in range(L)])
        in_maps.append({
            "xT": xTc.astype(ACT_NP),
            "wg": np.ascontiguousarray(wgc).astype(W_NP),
            "bg": np.ascontiguousarray(bgc).astype(np.float32),
        })
    return in_maps


def finish_output(results, order, B):
    out = np.empty((B, TOK, D), np.float32)
    for c in range(NCORES):
        outTc = results[c]["outT"]                  # [D, S*TOK] f32
        out[order[c * S:(c + 1) * S]] = outTc.T.reshape(S, TOK, D)
    return out


def kernel(x, cat_ids, W1, b1, W2, b2, W3, b3, W4, b4):
    global LAST_RESULT
    cat_ids = np.asarray(cat_ids).astype(np.int64)
    Ws = [np.asarray(w, dtype=np.float32) for w in (W1, W2, W3, W4)]
    bs = [np.asarray(b, dtype=np.float32) for b in (b1, b2, b3, b4)]
    x = np.asarray(x, dtype=np.float32)
    B = x.shape[0]

    order = _routing(cat_ids)
    in_maps = prepare_in_maps(x, cat_ids, Ws, bs, order)

    if "prog" not in _PROGRAM_CACHE:
        _PROGRAM_CACHE["prog"] = build_program()
    nc = _PROGRAM_CACHE["prog"]

    res = run_bass_kernel_spmd(nc, in_maps, list(range(NCORES)))
    LAST_RESULT = res
    return finish_output(res.results, order, B)


# revision 2
# speedup vs baseline: 3.2486x; 3.2486x over previous
"""Category-specific MLP (MoE-style routing) on 8 Trainium2 NeuronCores.

Strategy (expert-ish data parallel, host-routed):
  - Host sorts the 64 samples by cat_id and assigns 8 consecutive sorted
    samples to each of the 8 cores (perfect token balance: 2048 tok/core).
  - Host gathers each core's per-sample weight banks W_l[cat] into a
    per-core DRAM input, so the device kernel is a uniform SPMD program:
    8 sample slots x 4 dense layers of [256,1024]x[1024,1024].
  - Activations live in transposed layout [D, tok] on chip; each layer is
    out_T = W_l.T @ h_T computed as matmul(lhsT=W tile, rhs=h_T tile), so
    layers chain on the tensor engine with no transposes. Host transposes
    x once on the way in and the output once on the way out.
"""

import numpy as np
from contextlib import ExitStack

import concourse.bass as bass
import concourse.mybir as mybir
import concourse.tile as tile
from concourse import bacc
from concourse.bass_utils import run_bass_kernel_spmd

P = 128          # SBUF partitions
D = 1024         # model dim (in = hidden = out)
KT = D // P      # 8 k-tiles per dim
TOK = 256        # tokens per sample
S = 8            # sample slots per core
L = 4            # layers
NCORES = 8

import ml_dtypes

ACT_DT = mybir.dt.bfloat16  # on-chip activation dtype
W_DT = mybir.dt.bfloat16    # on-chip weight dtype
ACT_NP = ml_dtypes.bfloat16
W_NP = ml_dtypes.bfloat16

# Filled by kernel() with the BassKernelResults of the last run (for tests).
LAST_RESULT = None
_PROGRAM_CACHE = {}


def build_program(reps=1):
    """One SPMD program for all 8 cores: 8 slots x 4 layers.

    reps>1 wraps the whole computation in a hardware loop (only used for
    wall-clock slope timing in the test harness; grading uses reps=1).
    """
    nc = bacc.Bacc("TRN2", target_bir_lowering=False, debug=False,
                   num_devices=NCORES)
    xT = nc.dram_tensor("xT", [D, S * TOK], ACT_DT, kind="ExternalInput")
    wg = nc.dram_tensor("wg", [S, L, D, D], W_DT, kind="ExternalInput")
    bg = nc.dram_tensor("bg", [L, S, D], mybir.dt.float32, kind="ExternalInput")
    outT = nc.dram_tensor("outT", [D, S * TOK], mybir.dt.float32,
                          kind="ExternalOutput")

    xv = xT.ap().rearrange("(k p) n -> p k n", p=P)
    ov = outT.ap().rearrange("(k p) n -> p k n", p=P)
    bv = bg.ap().rearrange("l s (t p) -> p (l s t)", p=P)

    silu = mybir.ActivationFunctionType.Silu
    ident = mybir.ActivationFunctionType.Identity

    with tile.TileContext(nc) as tc, ExitStack() as ctx:
        wpool = ctx.enter_context(tc.tile_pool(name="w", bufs=3))
        hpool = ctx.enter_context(tc.tile_pool(name="h", bufs=3))
        opool = ctx.enter_context(tc.tile_pool(name="o", bufs=2))
        ppool = ctx.enter_context(tc.tile_pool(name="ps", bufs=6, space="PSUM"))
        cpool = ctx.enter_context(tc.tile_pool(name="c", bufs=1))

        btile = cpool.tile([P, L * S * KT], mybir.dt.float32)
        nc.sync.dma_start(btile[:], bv[:, :])

        def body(_iv=None):
            for s in range(S):
                hin = hpool.tile([P, KT, TOK], ACT_DT, tag="acts")
                nc.sync.dma_start(hin[:], xv[:, :, s * TOK:(s + 1) * TOK])
                for l in range(L):
                    w = wpool.tile([P, KT, D], W_DT, tag="w")
                    wsrc = wg.ap()[s, l].rearrange("(k p) m -> p k m", p=P)
                    for k in range(KT):
                        nc.sync.dma_start(w[:, k, :], wsrc[:, k, :])
                    last = l == L - 1
                    if last:
                        hout = opool.tile([P, KT, TOK], mybir.dt.float32,
                                          tag="outs")
                    else:
                        hout = hpool.tile([P, KT, TOK], ACT_DT, tag="acts")
                    for m in range(KT):
                        ps = ppool.tile([P, TOK], mybir.dt.float32)
                        for k in range(KT):
                            nc.tensor.matmul(ps[:], w[:, k, m * P:(m + 1) * P],
                                             hin[:, k, :],
                                             start=(k == 0), stop=(k == KT - 1))
                        col = (l * S + s) * KT + m
                        nc.scalar.activation(hout[:, m, :], ps[:],
                                             ident if last else silu,
                                             bias=btile[:, col:col + 1])
                    hin = hout
                nc.sync.dma_start(ov[:, :, s * TOK:(s + 1) * TOK], hin[:])

        if reps == 1:
            body()
        else:
            with tc.For_i(0, reps, 1) as iv:
                body(iv)
    nc.compile()
    return nc


def _routing(cat_ids):
    order = np.argsort(cat_ids, kind="stable")
    return order


def prepare_in_maps(x, cat_ids, Ws, bs, order):
    x = np.asarray(x)
    in_maps = []
    for c in range(NCORES):
        samp = order[c * S:(c + 1) * S]
        xs = np.asarray(x[samp], dtype=np.float32)          # [S, TOK, D]
        xTc = np.ascontiguousarray(xs.reshape(S * TOK, D).T)  # [D, S*TOK]
        cats = [int(cat_ids[i]) for i in samp]
        wgc = np.stack([np.stack([Ws[l][cat] for l in range(L)]) for cat in cats])
        bgc = np.stack([np.stack([bs[l][cat] for cat in cats]) for l in range(L)])
        in_maps.append({
            "xT": xTc.astype(ACT_NP),
            "wg": np.ascontiguousarray(wgc).astype(W_NP),
            "bg": np.ascontiguousarray(bgc).astype(np.float32),
        })
    return in_maps


def finish_output(results, order, B):
    out = np.empty((B, TOK, D), np.float32)
    for c in range(NCORES):
        outTc = results[c]["outT"]                  # [D, S*TOK] f32
        out[order[c * S:(c + 1) * S]] = outTc.T.reshape(S, TOK, D)
    return out


def kernel(x, cat_ids, W1, b1, W2, b2, W3, b3, W4, b4):
    global LAST_RESULT
    cat_ids = np.asarray(cat_ids).astype(np.int64)
    Ws = [np.asarray(w, dtype=np.float32) for w in (W1, W2, W3, W4)]
    bs = [np.asarray(b, dtype=np.float32) for b in (b1, b2, b3, b4)]
    x = np.asarray(x, dtype=np.float32)
    B = x.shape[0]

    order = _routing(cat_ids)
    in_maps = prepare_in_maps(x, cat_ids, Ws, bs, order)

    if "prog" not in _PROGRAM_CACHE:
        _PROGRAM_CACHE["prog"] = build_program()
    nc = _PROGRAM_CACHE["prog"]

    res = run_bass_kernel_spmd(nc, in_maps, list(range(NCORES)))
    LAST_RESULT = res
    return finish_output(res.results, order, B)


# revision 9
# speedup vs baseline: 3.9227x; 1.2075x over previous
"""Category-specific MLP (MoE-style routing) on 8 Trainium2 NeuronCores.

Strategy (host-routed expert/data parallel):
  - Host groups the 64 samples by cat_id into per-core work so every core
    gets exactly 8 samples (2048 tokens): token-balanced.
  - Same-cat samples are paired into 512-token "runs" where possible so a
    run does larger matmuls and loads its weight bank once. The per-core
    run profile (npair pair-runs + singles) is chosen from cat_ids and is
    identical on all cores, so one SPMD program serves all 8 cores; the
    program is (re)built per profile and cached.
  - Host gathers each run's weight bank W_l[cat] into a per-core DRAM
    input. Weights/activations are bf16 on chip (fp32 PSUM accumulate),
    final output fp32.
  - Activations live in transposed layout [D, tok]; each layer computes
    out_T = W_l.T @ h_T via matmul(lhsT=W tile, rhs=h_T tile), so layers
    chain on the tensor engine with no transposes. Host transposes x on
    the way in and the output on the way out.
"""

import numpy as np
from contextlib import ExitStack

import ml_dtypes

import concourse.bass as bass
import concourse.mybir as mybir
import concourse.tile as tile
from concourse import bacc
from concourse.bass_utils import run_bass_kernel_spmd

P = 128          # SBUF partitions
D = 1024         # model dim (in = hidden = out)
KT = D // P      # 8 k-tiles per dim
TOK = 256        # tokens per sample
S = 8            # samples per core
L = 4            # layers
NCORES = 8

ACT_DT = mybir.dt.bfloat16  # on-chip activation dtype
W_DT = mybir.dt.bfloat16    # on-chip weight dtype
ACT_NP = ml_dtypes.bfloat16
W_NP = ml_dtypes.bfloat16

# Filled by kernel() with the BassKernelResults of the last run (for tests).
LAST_RESULT = None
_PROGRAM_CACHE = {}


def plan(cat_ids):
    """Pick per-core sample order and the uniform run profile.

    Returns (order, npair): order is a [64] array of sample indices; core c
    owns order[8c:8c+8]. The first 2*npair samples of each core form npair
    same-cat pairs (512-token runs); the rest are single-sample runs.
    """
    cat_ids = np.asarray(cat_ids).astype(np.int64)
    by_cat = {}
    for i, c in enumerate(cat_ids.tolist()):
        by_cat.setdefault(c, []).append(i)
    pairs, singles = [], []
    for c in sorted(by_cat):
        lst = by_cat[c]
        for i in range(len(lst) // 2):
            pairs.append((lst[2 * i], lst[2 * i + 1]))
        if len(lst) % 2:
            singles.append(lst[-1])
    npair = min(S // 2, len(pairs) // NCORES)
    for a, b in pairs[npair * NCORES:]:
        singles.extend([a, b])
    pairs = pairs[:npair * NCORES]
    nsing = S - 2 * npair
    order = []
    for c in range(NCORES):
        for a, b in pairs[c * npair:(c + 1) * npair]:
            order.extend([a, b])
        order.extend(singles[c * nsing:(c + 1) * nsing])
    return np.asarray(order), npair


def _run_toks(npair):
    return [2 * TOK] * npair + [TOK] * (S - 2 * npair)


def build_program(reps=1, npair=0, mode="full", dual_dma=True):
    """One SPMD program for all 8 cores: R runs x 4 layers.

    reps>1 wraps the computation in a hardware loop (only used for
    wall-clock slope timing in the test harness; grading uses reps=1).
    mode: "full" (graded), "dma_only" / "compute_only" for bottleneck
    attribution in the test harness.
    """
    toks = _run_toks(npair)
    R = len(toks)
    offs = np.concatenate([[0], np.cumsum(toks)])

    nc = bacc.Bacc("TRN2", target_bir_lowering=False, debug=False,
                   num_devices=NCORES)
    xT = nc.dram_tensor("xT", [D, S * TOK], ACT_DT, kind="ExternalInput")
    wg = nc.dram_tensor("wg", [R, L, D, D], W_DT, kind="ExternalInput")
    bg = nc.dram_tensor("bg", [L, R, D], mybir.dt.float32, kind="ExternalInput")
    outT = nc.dram_tensor("outT", [D, S * TOK], mybir.dt.float32,
                          kind="ExternalOutput")

    xv = xT.ap().rearrange("(k p) n -> p k n", p=P)
    ov = outT.ap().rearrange("(k p) n -> p k n", p=P)
    bv = bg.ap().rearrange("l r (t p) -> p (l r t)", p=P)

    silu = mybir.ActivationFunctionType.Silu

    with tile.TileContext(nc) as tc, ExitStack() as ctx:
        wpool = ctx.enter_context(
            tc.tile_pool(name="w", bufs=5 if mode == "compute_only" else 3))
        hpool = ctx.enter_context(tc.tile_pool(name="h", bufs=3))
        opool = ctx.enter_context(tc.tile_pool(name="o", bufs=2))
        ppool = ctx.enter_context(tc.tile_pool(name="ps", bufs=6, space="PSUM"))
        cpool = ctx.enter_context(tc.tile_pool(name="c", bufs=1))

        btile = cpool.tile([P, L * R * KT], mybir.dt.float32)
        nc.sync.dma_start(btile[:], bv[:, :])

        def body(_iv=None):
            once_w = {}
            for r in range(R):
                tok, off = toks[r], int(offs[r])
                hin = hpool.tile([P, KT, 2 * TOK], ACT_DT, tag="acts")
                nc.sync.dma_start(hin[:, :, :tok], xv[:, :, off:off + tok])
                for l in range(L):
                    if mode == "compute_only" and l in once_w:
                        w = once_w[l]
                    else:
                        w = wpool.tile([P, KT, D], W_DT, tag="w")
                        wsrc = wg.ap()[r, l].rearrange("(k p) m -> p k m", p=P)
                        for k in range(KT):
                            eng = nc.scalar if (dual_dma and k % 2) else nc.sync
                            eng.dma_start(w[:, k, :], wsrc[:, k, :])
                        if mode == "compute_only":
                            once_w[l] = w
                    last = l == L - 1
                    if last:
                        hout = opool.tile([P, KT, 2 * TOK], mybir.dt.float32,
                                          tag="outs")
                    else:
                        hout = hpool.tile([P, KT, 2 * TOK], ACT_DT, tag="acts")
                    if mode == "dma_only":
                        continue
                    for m in range(KT):
                        ps = ppool.tile([P, 2 * TOK], mybir.dt.float32)
                        for k in range(KT):
                            nc.tensor.matmul(ps[:, :tok],
                                             w[:, k, m * P:(m + 1) * P],
                                             hin[:, k, :tok],
                                             start=(k == 0), stop=(k == KT - 1))
                        col = (l * R + r) * KT + m
                        if last:
                            # bias-add on DVE: keeps ACT running only Silu
                            # (no activation-table switches), f32 output.
                            nc.vector.tensor_scalar_add(
                                hout[:, m, :tok], ps[:, :tok],
                                btile[:, col:col + 1])
                        else:
                            nc.scalar.activation(hout[:, m, :tok], ps[:, :tok],
                                                 silu,
                                                 bias=btile[:, col:col + 1])
                    hin = hout
                if mode == "dma_only":
                    outsrc = hpool.tile([P, KT, 2 * TOK], mybir.dt.float32,
                                        tag="outs_f32")
                    nc.vector.tensor_copy(outsrc[:, 0, :tok], hin[:, 0, :tok])
                    nc.sync.dma_start(ov[:, :, off:off + tok],
                                      outsrc[:, :, :tok])
                else:
                    nc.sync.dma_start(ov[:, :, off:off + tok], hin[:, :, :tok])

        if reps == 1:
            body()
        else:
            with tc.For_i(0, reps, 1) as iv:
                body(iv)
    nc.compile()
    return nc


def prepare_in_maps(x, cat_ids, Ws, bs, order, npair):
    x = np.asarray(x)
    cat_ids = np.asarray(cat_ids).astype(np.int64)
    toks = _run_toks(npair)
    in_maps = []
    for c in range(NCORES):
        samp = order[c * S:(c + 1) * S]
        xs = np.asarray(x[samp], dtype=np.float32)            # [S, TOK, D]
        xTc = np.ascontiguousarray(xs.reshape(S * TOK, D).T)  # [D, S*TOK]
        # one weight bank per run; run r starts at sample index sum(prev)/TOK
        run_first = np.concatenate([[0], np.cumsum(toks)])[:-1] // TOK
        cats = [int(cat_ids[samp[i]]) for i in run_first]
        wgc = np.stack([np.stack([Ws[l][cat] for l in range(L)])
                        for cat in cats])                     # [R, L, D, D]
        bgc = np.stack([np.stack([bs[l][cat] for cat in cats])
                        for l in range(L)])                   # [L, R, D]
        in_maps.append({
            "xT": xTc.astype(ACT_NP),
            "wg": np.ascontiguousarray(wgc).astype(W_NP),
            "bg": np.ascontiguousarray(bgc).astype(np.float32),
        })
    return in_maps


def finish_output(results, order, B):
    out = np.empty((B, TOK, D), np.float32)
    for c in range(NCORES):
        outTc = results[c]["outT"]                  # [D, S*TOK] f32
        out[order[c * S:(c + 1) * S]] = outTc.T.reshape(S, TOK, D)
    return out


def kernel(x, cat_ids, W1, b1, W2, b2, W3, b3, W4, b4):
    global LAST_RESULT
    cat_ids = np.asarray(cat_ids).astype(np.int64)
    Ws = [np.asarray(w, dtype=np.float32) for w in (W1, W2, W3, W4)]
    bs = [np.asarray(b, dtype=np.float32) for b in (b1, b2, b3, b4)]
    x = np.asarray(x, dtype=np.float32)
    B = x.shape[0]

    order, npair = plan(cat_ids)
    in_maps = prepare_in_maps(x, cat_ids, Ws, bs, order, npair)

    if npair not in _PROGRAM_CACHE:
        _PROGRAM_CACHE[npair] = build_program(npair=npair)
    nc = _PROGRAM_CACHE[npair]

    res = run_bass_kernel_spmd(nc, in_maps, list(range(NCORES)))
    LAST_RESULT = res
    return finish_output(res.results, order, B)
